# revision 26
# baseline (speedup 1.0000x reference)
"""GAT-style GNN message passing on 8 TRN2 NeuronCores — no collectives.

Math: with LEAK=1 the leaky-relu is identity, so
  e[i,j,h] = e_src[i,h] + e_dst[j,h]
and softmax over j cancels e_src (and any row max) exactly:
  attn[i,j,h] = adj[i,j]*exp(e_dst[j,h]) / sum_j adj[i,j]*exp(e_dst[j,h])
  out[i,(h,f)] = (adj @ (z*h))[i,(h,f)] / (adj @ z)[i,h],  z = exp(e_dst)
then elu + log_softmax per row. log_softmax is shift invariant, so
elu(x) is computed as relu(x) + exp(min(x,0)) (drops the uniform -1).

Sharding: ROW-shard adj/out only; REPLICATE the h computation. Cross-core
paths were measured unusable here: the customcomms stack costs ~80us in
barriers, and a hand-rolled remote_dma_broadcast exchange delivers only
partially (large core start skew; cross-die D2D misroutes). So every
core loads the full x (fp8, 4MB) and computes h/z/G for all 4096 nodes
locally, then aggregates its own [512, 4096] adjacency slab.

v3 layout/pipeline changes over the 39.7us baseline:
 - Dual HWDGE rings (nc.sync + nc.scalar) stream concurrently at
   ~340GB/s aggregate vs ~220 on one ring.
 - The adjacency slab is split into two r-halves; each half has its own
   PSUM accumulation group, so half 0's evac + postprocess + store run
   while half 1's adjacency is still streaming.
 - zg work spread across engines: PSUM-touching ops on Vector/Scalar
   (alternating), SBUF-only z-copies on GpSimd (no PSUM port).

Precision: all matmul inputs are fp8 e4m3 (adj 0/1 exact; quantization
averages out over the 1024/2048-deep contractions; ~1.6e-3 end-to-end
vs the 2e-2 gate). W columns pre-scaled by 8 (fused a_dst columns by
32) against fp8-subnormal truncation; scales divided back out on chip.

Per-core device program (R = N/8 = 512 rows, P = 128):
  inputs:  xt [128, 8*8*512] fp8   xt[p, c*4096+k*512+n] = x[c*512+n, k*128+p]
           wt [128, 8*80]    fp8   wt[p, k*80+e] = w_ext[k*128+p, e] (e<72)
           at [128, 2*32*256] fp8  at[p, (u*32+j)*256+r]
                                     = adj[core*512+u*256+r, j*128+p]
  output:  out_p [128, 4*64] f32   out_p[p, q*64+f] = out[core*512+q*128+p, f]
"""

import sys

import numpy as np

if "/opt/trn_rl_repo" not in sys.path:
    sys.path.insert(0, "/opt/trn_rl_repo")

import ml_dtypes  # noqa: E402

import concourse.bass as bass  # noqa: E402
import concourse.tile as tile  # noqa: E402
from concourse import bacc, mybir  # noqa: E402
from concourse.bass_utils import run_bass_kernel_spmd  # noqa: E402
from concourse.masks import make_identity  # noqa: E402

N_CORES = 8
N_NODES = 4096
H = 8
F = 8
HF = H * F  # 64
EXT = HF + H  # 72: [h | e_dst]
EXTP = 80  # padded slot width (fp8 bytes) so DoubleRow strides are %16
K_IN = 1024
P = 128
KC = K_IN // P  # 8 k-chunks
CC = N_NODES // 512  # 8 column chunks for the h matmul
NC = N_NODES // P  # 32 j-chunks for the aggregation
R = N_NODES // N_CORES  # 512 rows per core
RC = R // P  # 4 output chunks per core
UH = 2  # aggregation r-halves
RU = R // UH  # 256 rows per half

S_W = 8.0  # host pre-scale on W columns (fp8 subnormal avoidance)
S_D = 32.0  # host pre-scale on the fused a_dst columns

N_WARMUP_MM = 8  # wide dummy matmuls to trip the PE HAM gate early

FP32 = mybir.dt.float32
BF16 = mybir.dt.bfloat16
FP8 = mybir.dt.float8e4
NP_FP8 = ml_dtypes.float8_e4m3
AFT = mybir.ActivationFunctionType
ALU = mybir.AluOpType
DR = mybir.MatmulPerfMode.DoubleRow


def _bcast_f(ap_pch):
    """[..., H] AP -> [..., H, F] AP broadcasting each head value over F."""
    return bass.AP(
        tensor=ap_pch.tensor,
        offset=ap_pch.offset,
        ap=list(ap_pch.ap) + [[0, F]],
    )


def build_bass() -> bass.Bass:
    nc = bacc.Bacc(num_devices=N_CORES)

    xt = nc.declare_dram_parameter("xt", [P, CC * KC * 512], FP8, isOutput=False)
    wt = nc.declare_dram_parameter("wt", [P, KC * EXTP], FP8, isOutput=False)
    at = nc.declare_dram_parameter("at", [P, UH * NC * RU], FP8, isOutput=False)
    out = nc.declare_dram_parameter("out", [P, RC * HF], FP32, isOutput=True)

    with tile.TileContext(nc) as tc:
        with (
            tc.tile_pool(name="singles", bufs=1) as singles,
            tc.tile_pool(name="hps", bufs=2, space="PSUM") as hps,
            tc.tile_pool(name="tps", bufs=1, space="PSUM") as tps,
            tc.tile_pool(name="aps", bufs=1, space="PSUM") as aps,
            tc.tile_pool(name="ops", bufs=1, space="PSUM") as ops,
            tc.tile_pool(name="work", bufs=2) as work,
        ):
            ident_bf = singles.tile([P, P], BF16)
            make_identity(nc, ident_bf)

            # --- loads: two HWDGE rings streaming concurrently. Each ring
            # carries half of xt (interleaved c-chunks so the h ladder
            # consumes in order), then half of the at r-half chunks.
            wt_sb = singles.tile([P, KC, EXTP], FP8)
            nc.sync.dma_start(
                out=wt_sb, in_=wt[:].rearrange("p (k e) -> p k e", k=KC)
            )
            xt_sb = singles.tile([P, CC, KC, 512], FP8)
            xt_view = xt[:].rearrange("p (c k n) -> p c k n", c=CC, k=KC)
            for c in range(0, CC, 2):
                nc.sync.dma_start(out=xt_sb[:, c : c + 1], in_=xt_view[:, c : c + 1])
                nc.scalar.dma_start(
                    out=xt_sb[:, c + 1 : c + 2], in_=xt_view[:, c + 1 : c + 2]
                )
            at_sb = singles.tile([P, UH, NC, RU], FP8)
            at_view = at[:].rearrange("p (u j r) -> p u j r", u=UH, j=NC)
            for u in range(UH):
                # j 0-15 on the sync ring in 256KB pieces so the
                # aggregation chases arrivals (SP self-paces on ring
                # credit harmlessly); j 16-31 on the scalar ring as ONE
                # 512KB piece per half, keeping the ACT queue at 6 DMA
                # issues total — a 7th+ would block ACT on ring credit
                # mid-stream and starve evacs/zg (measured ~12us stall).
                # (A SWDGE detour for the last piece was tried and
                # reverted: the scheduler hoists the dep-free Pool DMA
                # issue to the front, so it steals xt bandwidth early
                # instead of filling the ring-idle tail.)
                for j0 in range(0, 16, 8):
                    nc.sync.dma_start(
                        out=at_sb[:, u, j0 : j0 + 8], in_=at_view[:, u, j0 : j0 + 8]
                    )
                nc.scalar.dma_start(
                    out=at_sb[:, u, 16:32], in_=at_view[:, u, 16:32]
                )

            # Early throwaway Exp so the compiler's ACT_TABLE_LOAD lands
            # here (under the DMA/warmup window) instead of on the
            # critical path before the first real Exp.
            tbl = work.tile([1, 1], FP32, tag="tbl")
            nc.scalar.activation(tbl, ident_bf[0:1, 0:1], AFT.Exp)

            # --- postprocess PSUM tile (bf16 transposed output chunks) ---
            o_ps = ops.tile([P, RC, P], BF16)

            # --- PE warmup: wide matmuls on a zeroed scratch tile trip the
            # HAM activity window while the first xt DMA is in flight; the
            # aggregation group's first matmul clears the bank anyway.
            outT_ps = aps.tile([EXT, 512], FP32)
            if N_WARMUP_MM:
                warm_rhs = singles.tile([P, 512], BF16)
                nc.gpsimd.memset(warm_rhs, 0.0)
                for i in range(N_WARMUP_MM):
                    nc.tensor.matmul(
                        outT_ps[0:64, :],
                        lhsT=ident_bf[:, 0:64],
                        rhs=warm_rhs,
                        start=True,
                        stop=True,
                    )

            # --- hT = w_ext.T @ x.T : [72, 4096] fp8 matmuls, fp32 PSUM.
            # PE transposes trail the matmuls by two chunks; zg for each
            # quarter is emitted mid-loop so Scalar/Vector reach it as
            # soon as its data is ready. PSUM-touching zg ops alternate
            # Vector/Scalar; the SBUF-only z copy goes to GpSimd.
            hT_sb = singles.tile([EXT, CC, 512], BF16)
            tr_ps = tps.tile([P, NC, P], BF16)
            z_all = singles.tile([P, NC, H], BF16)
            g_ext = singles.tile([P, NC, EXTP], FP8)

            def do_transposes(c):
                for q in range(4):
                    j = c * 4 + q
                    nc.tensor.transpose(
                        tr_ps[:, j, :EXT],
                        hT_sb[:, c, q * P : (q + 1) * P],
                        ident_bf[:EXT, :EXT],
                    )

            def do_zg_range(sl):
                nc.scalar.activation(
                    z_all[:, sl, :], tr_ps[:, sl, HF:EXT], AFT.Exp, scale=1.0 / S_D
                )
                nc.vector.scalar_tensor_tensor(
                    out=g_ext[:, sl, 0:HF].rearrange("p c (h f) -> p c h f", h=H),
                    in0=tr_ps[:, sl, 0:HF].rearrange("p c (h f) -> p c h f", h=H),
                    scalar=1.0 / S_W,
                    in1=_bcast_f(z_all[:, sl, :]),
                    op0=ALU.mult,
                    op1=ALU.mult,
                )
                nc.gpsimd.tensor_copy(g_ext[:, sl, HF:EXT], z_all[:, sl, :])

            def do_zg(s):
                # one quarter: 8 j-chunks (two transposed hT chunks)
                do_zg_range(slice(8 * s, 8 * (s + 1)))

            for c in range(CC):
                hT_ps = hps.tile([EXT, 512], FP32, tag="hps", name=f"hT{c}")
                for t in range(KC // 2):
                    nc.tensor.matmul(
                        hT_ps,
                        lhsT=wt_sb[:, 2 * t : 2 * t + 2, :EXT],
                        rhs=xt_sb[:, c, 2 * t : 2 * t + 2, :],
                        start=(t == 0),
                        stop=(t == KC // 2 - 1),
                        perf_mode=DR,
                    )
                # evacuate to bf16 — evacs on Scalar so the Vector FIFO
                # holds only the zg stts + postprocess (the scheduler
                # hoists ready evacs ahead of stts within one engine's
                # stream, which serialized the g build when they shared
                # V). EXCEPT the last two chunks: V is idle right then,
                # and putting them on V cuts ~2 queue positions out of
                # the critical last-chunk evac->transpose->exp->stt chain.
                if c >= CC - 2:
                    nc.vector.tensor_copy(hT_sb[:, c, :], hT_ps)
                else:
                    nc.scalar.activation(hT_sb[:, c, :], hT_ps, AFT.Copy)
                if c >= 2:
                    do_transposes(c - 2)
                if c >= 3 and c % 2 == 1:
                    do_zg((c - 3) // 2)  # its transposed chunks just landed

            # split the last quarter so the final aggregation pairs
            # unblock as soon as their own transposes land
            do_transposes(CC - 2)
            do_zg_range(slice(24, 28))
            do_transposes(CC - 1)
            do_zg_range(slice(28, 32))

            # --- aggregation + postprocess per r-half: half u's 16 DR MMs
            # accumulate into columns [u*256, u*256+256) of outT_ps; its
            # evac/postprocess/store overlap half u+1's stream+MMs.
            out_sb = singles.tile([P, RC, HF], FP32)
            out_view = out[:].rearrange("p (q f) -> p q f", q=RC)
            outT_sb = singles.tile([EXT, 512], BF16)
            for u in range(UH):
                cols = slice(u * RU, (u + 1) * RU)
                for t in range(NC // 2):
                    nc.tensor.matmul(
                        outT_ps[:, cols],
                        lhsT=g_ext[:, 2 * t : 2 * t + 2, 0:EXT],
                        rhs=at_sb[:, u, 2 * t : 2 * t + 2, :],
                        start=(t == 0),
                        stop=(t == NC // 2 - 1),
                        perf_mode=DR,
                    )
                # evac this half (one engine per half; they pipeline)
                if u == 0:
                    nc.vector.tensor_copy(outT_sb[:, cols], outT_ps[:, cols])
                else:
                    nc.scalar.activation(outT_sb[:, cols], outT_ps[:, cols], AFT.Copy)

                # postprocess this half: x = num/den, elu+1, log_softmax
                sl = slice(2 * u, 2 * u + 2)
                for q in range(2 * u, 2 * u + 2):
                    nc.tensor.transpose(
                        o_ps[:, q, :EXT],
                        outT_sb[:, q * P : (q + 1) * P],
                        ident_bf[:EXT, :EXT],
                    )
                rd = work.tile([P, 2, H], FP32, tag="rd", name=f"rd{u}")
                nc.vector.reciprocal(rd, o_ps[:, sl, HF:EXT])
                xo = work.tile([P, 2, HF], FP32, tag="xo", name=f"xo{u}")
                nc.vector.tensor_mul(
                    xo[:].rearrange("p q (h f) -> p q h f", h=H),
                    o_ps[:, sl, 0:HF].rearrange("p q (h f) -> p q h f", h=H),
                    _bcast_f(rd[:]),
                )
                # y = relu(x) + min(exp(x), 1)  (= elu(x)+1; log_softmax
                # shift-safe). Exp-first so the two Vector ops run
                # back-to-back without a cross-engine semaphore hop.
                eo = work.tile([P, 2, HF], FP32, tag="eo", name=f"eo{u}")
                nc.scalar.activation(eo, xo, AFT.Exp)
                mo = work.tile([P, 2, HF], FP32, tag="mo", name=f"mo{u}")
                nc.vector.tensor_scalar_min(mo, eo, 1.0)
                yo = work.tile([P, 2, HF], FP32, tag="yo", name=f"yo{u}")
                nc.vector.scalar_tensor_tensor(
                    out=yo, in0=xo, scalar=0.0, in1=mo, op0=ALU.max, op1=ALU.add
                )
                ex = work.tile([P, 2, HF], FP32, tag="ex", name=f"ex{u}")
                nc.scalar.activation(ex, yo, AFT.Exp)
                sm = work.tile([P, 2], FP32, tag="sm", name=f"sm{u}")
                nc.vector.reduce_sum(sm, ex, axis=mybir.AxisListType.X)
                ls = work.tile([P, 2], FP32, tag="ls", name=f"ls{u}")
                nc.scalar.activation(ls, sm, AFT.Ln)
                ls_b = bass.AP(
                    tensor=ls[:].tensor,
                    offset=ls[:].offset,
                    ap=list(ls[:].ap) + [[0, HF]],
                )
                nc.vector.tensor_sub(out_sb[:, sl], yo, ls_b)
                nc.sync.dma_start(out=out_view[:, sl], in_=out_sb[:, sl])

    # Force all ACT activations (Exp + Ln) onto the one table set containing
    # both, so only ONE ACT_TABLE_LOAD is emitted (early, hidden under DMA)
    # instead of a ~1.3us reload at every Exp<->Ln switch.
    orig_gat = bacc.get_activation_tables

    def _one_set(arch):
        return {
            k: (v if k == "natural_log_exp_and_others" else set())
            for k, v in orig_gat(arch).items()
        }

    bacc.get_activation_tables = _one_set
    try:
        nc.finalize()
    finally:
        bacc.get_activation_tables = orig_gat
    return nc


def _host_prep(x, adj, W, a_dst):
    """Build per-core input maps (xt/wt replicated, at row-sharded)."""
    Wd = np.einsum(
        "khf,hf->kh", W.reshape(K_IN, H, F), a_dst, dtype=np.float32
    ).astype(np.float32)
    w_ext = np.concatenate([W * S_W, Wd * S_D], axis=1)  # [1024, 72]
    wt_np = np.zeros((P, KC, EXTP), dtype=NP_FP8)
    wt_np[:, :, :EXT] = (
        w_ext.reshape(KC, P, EXT).transpose(1, 0, 2).astype(NP_FP8)
    )
    wt_np = wt_np.reshape(P, KC * EXTP)

    x8 = x.astype(NP_FP8)  # [4096, 1024]
    # xt[p, c, k, n] = x8[c*512+n, k*128+p]
    xt_np = np.ascontiguousarray(
        x8.reshape(CC, 512, KC, P).transpose(3, 0, 2, 1)
    ).reshape(P, CC * KC * 512)

    adj8 = (adj > 0).astype(NP_FP8)  # [4096, 4096]
    in_maps = []
    for c in range(N_CORES):
        rows = slice(c * R, (c + 1) * R)
        # at[p, u, j, r] = adj8[c*512 + u*256 + r, j*128+p]
        at_np = np.ascontiguousarray(
            adj8[rows].reshape(UH, RU, NC, P).transpose(3, 0, 2, 1)
        ).reshape(P, UH * NC * RU)
        in_maps.append({"xt": xt_np, "wt": wt_np, "at": at_np})
    return in_maps


_BUILT = {}


def run(x, adj, W, a_dst, trace=False):
    if "nc" not in _BUILT:
        _BUILT["nc"] = build_bass()
    nc = _BUILT["nc"]
    in_maps = _host_prep(x, adj, W, a_dst)
    res = run_bass_kernel_spmd(nc, in_maps, list(range(N_CORES)), trace=trace)
    blocks = []
    for c in range(N_CORES):
        o = res.results[c]["out"]  # [P, RC*HF] p-major
        blocks.append(o.reshape(P, RC, HF).transpose(1, 0, 2).reshape(R, HF))
    return np.concatenate(blocks, axis=0).astype(np.float32), res


def kernel(x, adj, W, a_src, a_dst):
    x = np.asarray(x, dtype=np.float32)
    adj = np.asarray(adj)
    W = np.asarray(W, dtype=np.float32)
    a_dst = np.asarray(a_dst, dtype=np.float32)
    out, _ = run(x, adj, W, a_dst, trace=False)
    return out


# revision 27
# speedup vs baseline: 1.0818x; 1.0818x over previous
"""GAT-style GNN message passing on 8 TRN2 NeuronCores — no collectives.

Math: with LEAK=1 the leaky-relu is identity, so
  e[i,j,h] = e_src[i,h] + e_dst[j,h]
and softmax over j cancels e_src (and any row max) exactly:
  attn[i,j,h] = adj[i,j]*exp(e_dst[j,h]) / sum_j adj[i,j]*exp(e_dst[j,h])
  out[i,(h,f)] = (adj @ (z*h))[i,(h,f)] / (adj @ z)[i,h],  z = exp(e_dst)
then elu + log_softmax per row. log_softmax is shift invariant, so
elu(x) is computed as relu(x) + exp(min(x,0)) (drops the uniform -1).

Sharding: ROW-shard adj/out only; REPLICATE the h computation. Cross-core
paths were measured unusable here: the customcomms stack costs ~80us in
barriers, and a hand-rolled remote_dma_broadcast exchange delivers only
partially (large core start skew; cross-die D2D misroutes). So every
core loads the full x (fp8, 4MB) and computes h/z/G for all 4096 nodes
locally, then aggregates its own [512, 4096] adjacency slab.

v3 layout/pipeline changes over the 39.7us baseline:
 - Dual HWDGE rings (nc.sync + nc.scalar) stream concurrently at
   ~340GB/s aggregate vs ~220 on one ring.
 - The adjacency slab is split into two r-halves; each half has its own
   PSUM accumulation group, so half 0's evac + postprocess + store run
   while half 1's adjacency is still streaming.
 - zg work spread across engines: PSUM-touching ops on Vector/Scalar
   (alternating), SBUF-only z-copies on GpSimd (no PSUM port).

Precision: all matmul inputs are fp8 e4m3 (adj 0/1 exact; quantization
averages out over the 1024/2048-deep contractions; ~1.6e-3 end-to-end
vs the 2e-2 gate). W columns pre-scaled by 8 (fused a_dst columns by
32) against fp8-subnormal truncation; scales divided back out on chip.

Per-core device program (R = N/8 = 512 rows, P = 128):
  inputs:  xt [128, 8*8*512] fp8   xt[p, c*4096+k*512+n] = x[c*512+n, k*128+p]
           wt [128, 8*80]    fp8   wt[p, k*80+e] = w_ext[k*128+p, e] (e<72)
           at [128, 2*32*256] fp8  at[p, (u*32+j)*256+r]
                                     = adj[core*512+u*256+r, j*128+p]
  output:  out_p [128, 4*64] f32   out_p[p, q*64+f] = out[core*512+q*128+p, f]
"""

import sys

import numpy as np

if "/opt/trn_rl_repo" not in sys.path:
    sys.path.insert(0, "/opt/trn_rl_repo")

import ml_dtypes  # noqa: E402

import concourse.bass as bass  # noqa: E402
import concourse.tile as tile  # noqa: E402
from concourse import bacc, mybir  # noqa: E402
from concourse.bass_utils import run_bass_kernel_spmd  # noqa: E402
from concourse.masks import make_identity  # noqa: E402

N_CORES = 8
N_NODES = 4096
H = 8
F = 8
HF = H * F  # 64
EXT = HF + H  # 72: [h | e_dst]
EXTP = 80  # padded slot width (fp8 bytes) so DoubleRow strides are %16
K_IN = 1024
P = 128
KC = K_IN // P  # 8 k-chunks
CC = N_NODES // 512  # 8 column chunks for the h matmul
NC = N_NODES // P  # 32 j-chunks for the aggregation
R = N_NODES // N_CORES  # 512 rows per core
RC = R // P  # 4 output chunks per core
UH = 2  # aggregation r-halves
RU = R // UH  # 256 rows per half

S_W = 8.0  # host pre-scale on W columns (fp8 subnormal avoidance)
S_D = 32.0  # host pre-scale on the fused a_dst columns

N_WARMUP_MM = 8  # wide dummy matmuls to trip the PE HAM gate early

FP32 = mybir.dt.float32
BF16 = mybir.dt.bfloat16
FP8 = mybir.dt.float8e4
NP_FP8 = ml_dtypes.float8_e4m3
AFT = mybir.ActivationFunctionType
ALU = mybir.AluOpType
DR = mybir.MatmulPerfMode.DoubleRow


def _bcast_f(ap_pch):
    """[..., H] AP -> [..., H, F] AP broadcasting each head value over F."""
    return bass.AP(
        tensor=ap_pch.tensor,
        offset=ap_pch.offset,
        ap=list(ap_pch.ap) + [[0, F]],
    )


def build_bass() -> bass.Bass:
    nc = bacc.Bacc(num_devices=N_CORES)

    xt = nc.declare_dram_parameter("xt", [P, CC * KC * 512], FP8, isOutput=False)
    wt = nc.declare_dram_parameter("wt", [P, KC * EXTP], FP8, isOutput=False)
    at = nc.declare_dram_parameter("at", [P, UH * NC * RU], FP8, isOutput=False)
    out = nc.declare_dram_parameter("out", [P, RC * HF], FP32, isOutput=True)

    with tile.TileContext(nc) as tc:
        with (
            tc.tile_pool(name="singles", bufs=1) as singles,
            tc.tile_pool(name="hps", bufs=2, space="PSUM") as hps,
            tc.tile_pool(name="tps", bufs=1, space="PSUM") as tps,
            tc.tile_pool(name="aps", bufs=1, space="PSUM") as aps,
            tc.tile_pool(name="ops", bufs=1, space="PSUM") as ops,
            tc.tile_pool(name="work", bufs=2) as work,
        ):
            ident_bf = singles.tile([P, P], BF16)
            make_identity(nc, ident_bf)

            # --- loads: two HWDGE rings streaming concurrently. Each ring
            # carries half of xt (interleaved c-chunks so the h ladder
            # consumes in order), then half of the at r-half chunks.
            wt_sb = singles.tile([P, KC, EXTP], FP8)
            nc.sync.dma_start(
                out=wt_sb, in_=wt[:].rearrange("p (k e) -> p k e", k=KC)
            )
            xt_sb = singles.tile([P, CC, KC, 512], FP8)
            xt_view = xt[:].rearrange("p (c k n) -> p c k n", c=CC, k=KC)
            for c in range(0, CC, 2):
                nc.sync.dma_start(out=xt_sb[:, c : c + 1], in_=xt_view[:, c : c + 1])
                nc.scalar.dma_start(
                    out=xt_sb[:, c + 1 : c + 2], in_=xt_view[:, c + 1 : c + 2]
                )
            at_sb = singles.tile([P, UH, NC, RU], FP8)
            at_view = at[:].rearrange("p (u j r) -> p u j r", u=UH, j=NC)
            for u in range(UH):
                # j 0-15 on the sync ring in 256KB pieces so the
                # aggregation chases arrivals (SP self-paces on ring
                # credit harmlessly); j 16-31 on the scalar ring as ONE
                # 512KB piece per half, keeping the ACT queue at 6 DMA
                # issues total — a 7th+ would block ACT on ring credit
                # mid-stream and starve evacs/zg (measured ~12us stall).
                # (A SWDGE detour for the last piece was tried and
                # reverted: the scheduler hoists the dep-free Pool DMA
                # issue to the front, so it steals xt bandwidth early
                # instead of filling the ring-idle tail.)
                for j0 in range(0, 16, 8):
                    nc.sync.dma_start(
                        out=at_sb[:, u, j0 : j0 + 8], in_=at_view[:, u, j0 : j0 + 8]
                    )
                nc.scalar.dma_start(
                    out=at_sb[:, u, 16:32], in_=at_view[:, u, 16:32]
                )

            # Early throwaway Exp so the compiler's ACT_TABLE_LOAD lands
            # here (under the DMA/warmup window) instead of on the
            # critical path before the first real Exp.
            tbl = work.tile([1, 1], FP32, tag="tbl")
            nc.scalar.activation(tbl, ident_bf[0:1, 0:1], AFT.Exp)

            # --- postprocess PSUM tile (bf16 transposed output chunks) ---
            o_ps = ops.tile([P, RC, P], BF16)

            # --- PE warmup: wide matmuls on a zeroed scratch tile trip the
            # HAM activity window while the first xt DMA is in flight; the
            # aggregation group's first matmul clears the bank anyway.
            outT_ps = aps.tile([EXT, 512], FP32)
            if N_WARMUP_MM:
                warm_rhs = singles.tile([P, 512], BF16)
                nc.gpsimd.memset(warm_rhs, 0.0)
                for i in range(N_WARMUP_MM):
                    nc.tensor.matmul(
                        outT_ps[0:64, :],
                        lhsT=ident_bf[:, 0:64],
                        rhs=warm_rhs,
                        start=True,
                        stop=True,
                    )

            # --- hT = w_ext.T @ x.T : [72, 4096] fp8 matmuls, fp32 PSUM.
            # PE transposes trail the matmuls by two chunks; zg for each
            # quarter is emitted mid-loop so Scalar/Vector reach it as
            # soon as its data is ready. PSUM-touching zg ops alternate
            # Vector/Scalar; the SBUF-only z copy goes to GpSimd.
            hT_sb = singles.tile([EXT, CC, 512], BF16)
            tr_ps = tps.tile([P, NC, P], BF16)
            z_all = singles.tile([P, NC, H], BF16)
            g_ext = singles.tile([P, NC, EXTP], FP8)

            def do_transposes(c):
                for q in range(4):
                    j = c * 4 + q
                    nc.tensor.transpose(
                        tr_ps[:, j, :EXT],
                        hT_sb[:, c, q * P : (q + 1) * P],
                        ident_bf[:EXT, :EXT],
                    )

            def do_zg_range(sl):
                nc.scalar.activation(
                    z_all[:, sl, :], tr_ps[:, sl, HF:EXT], AFT.Exp, scale=1.0 / S_D
                )
                nc.vector.scalar_tensor_tensor(
                    out=g_ext[:, sl, 0:HF].rearrange("p c (h f) -> p c h f", h=H),
                    in0=tr_ps[:, sl, 0:HF].rearrange("p c (h f) -> p c h f", h=H),
                    scalar=1.0 / S_W,
                    in1=_bcast_f(z_all[:, sl, :]),
                    op0=ALU.mult,
                    op1=ALU.mult,
                )
                nc.gpsimd.tensor_copy(g_ext[:, sl, HF:EXT], z_all[:, sl, :])

            def do_zg(s):
                # one quarter: 8 j-chunks (two transposed hT chunks)
                do_zg_range(slice(8 * s, 8 * (s + 1)))

            for c in range(CC):
                hT_ps = hps.tile([EXT, 512], FP32, tag="hps", name=f"hT{c}")
                for t in range(KC // 2):
                    nc.tensor.matmul(
                        hT_ps,
                        lhsT=wt_sb[:, 2 * t : 2 * t + 2, :EXT],
                        rhs=xt_sb[:, c, 2 * t : 2 * t + 2, :],
                        start=(t == 0),
                        stop=(t == KC // 2 - 1),
                        perf_mode=DR,
                    )
                # evacuate to bf16 — ALL evacs on Scalar so the Vector
                # FIFO holds only the zg stts + postprocess: the scheduler
                # hoists ready evacs ahead of stts within one engine's
                # stream, which serialized the g build when they shared V.
                nc.scalar.activation(hT_sb[:, c, :], hT_ps, AFT.Copy)
                if c >= 2:
                    do_transposes(c - 2)
                if c >= 3 and c % 2 == 1:
                    do_zg((c - 3) // 2)  # its transposed chunks just landed

            # split the last quarter so the final aggregation pairs
            # unblock as soon as their own transposes land
            do_transposes(CC - 2)
            do_zg_range(slice(24, 28))
            do_transposes(CC - 1)
            do_zg_range(slice(28, 32))

            # --- aggregation + postprocess per r-half: half u's 16 DR MMs
            # accumulate into columns [u*256, u*256+256) of outT_ps; its
            # evac/postprocess/store overlap half u+1's stream+MMs.
            out_sb = singles.tile([P, RC, HF], FP32)
            out_view = out[:].rearrange("p (q f) -> p q f", q=RC)
            outT_sb = singles.tile([EXT, 512], BF16)
            for u in range(UH):
                cols = slice(u * RU, (u + 1) * RU)
                for t in range(NC // 2):
                    nc.tensor.matmul(
                        outT_ps[:, cols],
                        lhsT=g_ext[:, 2 * t : 2 * t + 2, 0:EXT],
                        rhs=at_sb[:, u, 2 * t : 2 * t + 2, :],
                        start=(t == 0),
                        stop=(t == NC // 2 - 1),
                        perf_mode=DR,
                    )
                # evac this half (one engine per half; they pipeline)
                if u == 0:
                    nc.vector.tensor_copy(outT_sb[:, cols], outT_ps[:, cols])
                else:
                    nc.scalar.activation(outT_sb[:, cols], outT_ps[:, cols], AFT.Copy)

                # postprocess this half: x = num/den, elu+1, log_softmax
                sl = slice(2 * u, 2 * u + 2)
                for q in range(2 * u, 2 * u + 2):
                    nc.tensor.transpose(
                        o_ps[:, q, :EXT],
                        outT_sb[:, q * P : (q + 1) * P],
                        ident_bf[:EXT, :EXT],
                    )
                rd = work.tile([P, 2, H], FP32, tag="rd", name=f"rd{u}")
                nc.vector.reciprocal(rd, o_ps[:, sl, HF:EXT])
                xo = work.tile([P, 2, HF], FP32, tag="xo", name=f"xo{u}")
                nc.vector.tensor_mul(
                    xo[:].rearrange("p q (h f) -> p q h f", h=H),
                    o_ps[:, sl, 0:HF].rearrange("p q (h f) -> p q h f", h=H),
                    _bcast_f(rd[:]),
                )
                # y = relu(x) + min(exp(x), 1)  (= elu(x)+1; log_softmax
                # shift-safe). Exp-first so the two Vector ops run
                # back-to-back without a cross-engine semaphore hop.
                eo = work.tile([P, 2, HF], FP32, tag="eo", name=f"eo{u}")
                nc.scalar.activation(eo, xo, AFT.Exp)
                mo = work.tile([P, 2, HF], FP32, tag="mo", name=f"mo{u}")
                nc.vector.tensor_scalar_min(mo, eo, 1.0)
                yo = work.tile([P, 2, HF], FP32, tag="yo", name=f"yo{u}")
                nc.vector.scalar_tensor_tensor(
                    out=yo, in0=xo, scalar=0.0, in1=mo, op0=ALU.max, op1=ALU.add
                )
                ex = work.tile([P, 2, HF], FP32, tag="ex", name=f"ex{u}")
                nc.scalar.activation(ex, yo, AFT.Exp)
                sm = work.tile([P, 2], FP32, tag="sm", name=f"sm{u}")
                nc.vector.reduce_sum(sm, ex, axis=mybir.AxisListType.X)
                ls = work.tile([P, 2], FP32, tag="ls", name=f"ls{u}")
                nc.scalar.activation(ls, sm, AFT.Ln)
                ls_b = bass.AP(
                    tensor=ls[:].tensor,
                    offset=ls[:].offset,
                    ap=list(ls[:].ap) + [[0, HF]],
                )
                nc.vector.tensor_sub(out_sb[:, sl], yo, ls_b)
                nc.sync.dma_start(out=out_view[:, sl], in_=out_sb[:, sl])

    # Force all ACT activations (Exp + Ln) onto the one table set containing
    # both, so only ONE ACT_TABLE_LOAD is emitted (early, hidden under DMA)
    # instead of a ~1.3us reload at every Exp<->Ln switch.
    orig_gat = bacc.get_activation_tables

    def _one_set(arch):
        return {
            k: (v if k == "natural_log_exp_and_others" else set())
            for k, v in orig_gat(arch).items()
        }

    bacc.get_activation_tables = _one_set
    try:
        nc.finalize()
    finally:
        bacc.get_activation_tables = orig_gat
    return nc


def _host_prep(x, adj, W, a_dst):
    """Build per-core input maps (xt/wt replicated, at row-sharded)."""
    Wd = np.einsum(
        "khf,hf->kh", W.reshape(K_IN, H, F), a_dst, dtype=np.float32
    ).astype(np.float32)
    w_ext = np.concatenate([W * S_W, Wd * S_D], axis=1)  # [1024, 72]
    wt_np = np.zeros((P, KC, EXTP), dtype=NP_FP8)
    wt_np[:, :, :EXT] = (
        w_ext.reshape(KC, P, EXT).transpose(1, 0, 2).astype(NP_FP8)
    )
    wt_np = wt_np.reshape(P, KC * EXTP)

    x8 = x.astype(NP_FP8)  # [4096, 1024]
    # xt[p, c, k, n] = x8[c*512+n, k*128+p]
    xt_np = np.ascontiguousarray(
        x8.reshape(CC, 512, KC, P).transpose(3, 0, 2, 1)
    ).reshape(P, CC * KC * 512)

    adj8 = (adj > 0).astype(NP_FP8)  # [4096, 4096]
    in_maps = []
    for c in range(N_CORES):
        rows = slice(c * R, (c + 1) * R)
        # at[p, u, j, r] = adj8[c*512 + u*256 + r, j*128+p]
        at_np = np.ascontiguousarray(
            adj8[rows].reshape(UH, RU, NC, P).transpose(3, 0, 2, 1)
        ).reshape(P, UH * NC * RU)
        in_maps.append({"xt": xt_np, "wt": wt_np, "at": at_np})
    return in_maps


_BUILT = {}


def run(x, adj, W, a_dst, trace=False):
    if "nc" not in _BUILT:
        _BUILT["nc"] = build_bass()
    nc = _BUILT["nc"]
    in_maps = _host_prep(x, adj, W, a_dst)
    res = run_bass_kernel_spmd(nc, in_maps, list(range(N_CORES)), trace=trace)
    blocks = []
    for c in range(N_CORES):
        o = res.results[c]["out"]  # [P, RC*HF] p-major
        blocks.append(o.reshape(P, RC, HF).transpose(1, 0, 2).reshape(R, HF))
    return np.concatenate(blocks, axis=0).astype(np.float32), res


def kernel(x, adj, W, a_src, a_dst):
    x = np.asarray(x, dtype=np.float32)
    adj = np.asarray(adj)
    W = np.asarray(W, dtype=np.float32)
    a_dst = np.asarray(a_dst, dtype=np.float32)
    out, _ = run(x, adj, W, a_dst, trace=False)
    return out
